# revision 1
# baseline (speedup 1.0000x reference)
"""DEDICOM decoder forward on 8 Trainium2 NeuronCores.

Math per relation k (k=0..7):
    M_k = diag(d_k) @ G @ diag(d_k)                  (64x64, host-precomputed)
    out[k, n] = sigmoid( (row_n @ M_k) . col_n )

Device algorithm (data-parallel over N across 8 cores; per core SHARD=62500
samples padded to 489*128):
  - samples live 128-per-tile on SBUF partitions (partition p holds a
    contiguous HBM chunk so input DMA descriptors are fully contiguous)
  - PE: transpose row tile [128,64] -> [64,128] (bf16), then
        T = rowT.T @ M_all  with M_all = [64, 8*64] stacked M_k  -> PSUM fp32
  - ACT: cast T -> bf16 SBUF
  - DVE: U = T * broadcast(col) ; rec[:,k] = sum_j U[:,k,j]
  - ACT: sigmoid
  - DMA out [shard, 8] fp32; host reassembles/transposes to [8, N]
"""

import sys

sys.path.insert(0, "/opt/trn_rl_repo")

import numpy as np
import ml_dtypes

import concourse.bass as bass
import concourse.bacc as bacc
import concourse.mybir as mybir
from concourse import tile
from concourse.bass_utils import run_bass_kernel_spmd

N, D, R = 500000, 64, 8
NCORES = 8
SHARD = N // NCORES            # 62500
TPP = 490                      # samples per partition; 490*128 = 62720 >= 62500
SHARD_PAD = TPP * 128
W = 70                         # group width (samples/partition/group); 7 groups
NGROUPS = TPP // W
BF16 = mybir.dt.bfloat16
F32 = mybir.dt.float32

_CACHE: dict = {}


def _build_program():
    if "nc" in _CACHE:
        return _CACHE["nc"]

    nc = bacc.Bacc(
        "TRN2", target_bir_lowering=False, debug=False, num_devices=NCORES
    )

    row_d = nc.dram_tensor("row", [SHARD_PAD, D], F32, kind="ExternalInput")
    col_d = nc.dram_tensor("col", [SHARD_PAD, D], F32, kind="ExternalInput")
    mall_d = nc.dram_tensor("mall", [D, R * D], F32, kind="ExternalInput")
    ident_d = nc.dram_tensor("ident", [128, 128], BF16, kind="ExternalInput")
    out_d = nc.dram_tensor("out", [SHARD_PAD, R], F32, kind="ExternalOutput")

    row_v = row_d.ap().rearrange("(p t) d -> p t d", p=128)
    col_v = col_d.ap().rearrange("(p t) d -> p t d", p=128)
    out_v = out_d.ap().rearrange("(p t) k -> p t k", p=128)

    X = mybir.AxisListType.X
    ADD = mybir.AluOpType.add
    MULT = mybir.AluOpType.mult

    with tile.TileContext(nc) as tc:
        with (
            tc.tile_pool(name="const", bufs=1) as cpool,
            tc.tile_pool(name="io", bufs=2) as iopool,
            tc.tile_pool(name="work", bufs=4) as wpool,
            tc.tile_pool(name="psum_t", bufs=3, space="PSUM") as pt_pool,
            tc.tile_pool(name="psum_r", bufs=2, space="PSUM") as pr_pool,
        ):
            mall = cpool.tile([D, R * D], BF16, tag="mall")
            ident = cpool.tile([128, 128], BF16, tag="ident")
            nc.gpsimd.dma_start(mall[:], mall_d.ap())  # casts f32 -> bf16
            nc.sync.dma_start(ident[:], ident_d.ap())

            for g in range(NGROUPS):
                t0 = g * W
                row_g = iopool.tile([128, W, D], BF16, tag="row_g")
                col_g = iopool.tile([128, W, D], BF16, tag="col_g")
                nc.gpsimd.dma_start(row_g[:], row_v[:, t0 : t0 + W, :])
                nc.gpsimd.dma_start(col_g[:], col_v[:, t0 : t0 + W, :])

                rec_g = wpool.tile([128, W, R], F32, tag="rec")

                for b0 in range(0, W, 8):
                    bw = min(8, W - b0)
                    rowT_ps = pr_pool.tile([64, 8, 128], BF16, tag="rowT")
                    rowT_sb = wpool.tile([64, 8, 128], BF16, tag="rowT_sb")
                    for i in range(bw):
                        t = b0 + i
                        nc.tensor.transpose(
                            rowT_ps[:, i, :], row_g[:, t, :], ident[:]
                        )
                    nc.scalar.copy(rowT_sb[:, :bw, :], rowT_ps[:, :bw, :])
                    for i in range(bw):
                        t = b0 + i
                        T_ps = pt_pool.tile([128, R * D], F32, tag="T")
                        nc.tensor.matmul(T_ps[:], rowT_sb[:, i, :], mall[:])
                        T_sb = wpool.tile([128, R, D], BF16, tag="T_sb")
                        nc.scalar.copy(
                            T_sb[:].rearrange("p k j -> p (k j)"), T_ps[:]
                        )
                        U = wpool.tile([128, R, D], BF16, tag="U")
                        colb = (
                            col_g[:, t, :]
                            .unsqueeze(1)
                            .broadcast_to([128, R, D])
                        )
                        nc.vector.tensor_tensor(
                            out=U[:], in0=T_sb[:], in1=colb, op=MULT
                        )
                        # pairwise-fold the 64-wide reduction with TT adds
                        # (~2x faster/elem than TENSOR_REDUCE), then reduce 16
                        U2 = wpool.tile([128, R, 32], BF16, tag="U2")
                        nc.vector.tensor_tensor(
                            out=U2[:], in0=U[:, :, 0:32],
                            in1=U[:, :, 32:64], op=ADD,
                        )
                        U3 = wpool.tile([128, R, 16], BF16, tag="U3")
                        nc.vector.tensor_tensor(
                            out=U3[:], in0=U2[:, :, 0:16],
                            in1=U2[:, :, 16:32], op=ADD,
                        )
                        nc.vector.tensor_reduce(
                            rec_g[:, t, :], U3[:], axis=X, op=ADD
                        )

                sig_g = wpool.tile([128, W, R], F32, tag="sig")
                nc.scalar.activation(
                    sig_g[:],
                    rec_g[:],
                    mybir.ActivationFunctionType.Sigmoid,
                )
                nc.sync.dma_start(out_v[:, t0 : t0 + W, :], sig_g[:])

    nc.compile()
    _CACHE["nc"] = nc
    return nc


def _prep_inputs(inputs_row, inputs_col, global_interaction, local_variation):
    d = np.asarray(local_variation, np.float32)
    g = np.asarray(global_interaction, np.float32)
    # M_all[i, (k, j)] = d[k, i] * G[i, j] * d[k, j]
    mall = np.einsum("ki,ij,kj->ikj", d, g, d).reshape(D, R * D)
    mall = np.ascontiguousarray(mall, np.float32)
    ident = np.eye(128, dtype=ml_dtypes.bfloat16)

    pad = SHARD_PAD - SHARD
    in_maps = []
    for c in range(NCORES):
        sl = slice(c * SHARD, (c + 1) * SHARD)
        rr = np.concatenate(
            [np.asarray(inputs_row[sl], np.float32), np.zeros((pad, D), np.float32)]
        )
        cc = np.concatenate(
            [np.asarray(inputs_col[sl], np.float32), np.zeros((pad, D), np.float32)]
        )
        in_maps.append(
            {
                "row": np.ascontiguousarray(rr),
                "col": np.ascontiguousarray(cc),
                "mall": mall,
                "ident": ident,
            }
        )
    return in_maps


def kernel(inputs_row, inputs_col, global_interaction, local_variation):
    nc = _build_program()
    in_maps = _prep_inputs(
        inputs_row, inputs_col, global_interaction, local_variation
    )
    res = run_bass_kernel_spmd(nc, in_maps, list(range(NCORES)))
    outs = [res.results[c]["out"][:SHARD] for c in range(NCORES)]
    full = np.concatenate(outs, axis=0)  # [N, 8] f32
    return np.ascontiguousarray(full.T)  # [8, N]


if __name__ == "__main__":
    rng = np.random.default_rng(0)
    inputs = {
        "inputs_row": rng.standard_normal((N, D), dtype=np.float32),
        "inputs_col": rng.standard_normal((N, D), dtype=np.float32),
        "global_interaction": rng.uniform(-0.2, 0.2, (D, D)).astype(np.float32),
        "local_variation": rng.uniform(-0.3, 0.3, (R, D)).astype(np.float32),
    }
    out = kernel(**inputs)
    print("out", out.shape, out.dtype, out[:, :3])



# revision 4
# speedup vs baseline: 1.1677x; 1.1677x over previous
"""DEDICOM decoder forward on 8 Trainium2 NeuronCores.

Math per relation k (k=0..7):
    M_k = diag(d_k) @ G @ diag(d_k)                  (64x64, host-precomputed)
    out[k, n] = sigmoid( (row_n @ M_k) . col_n )

Device algorithm (data-parallel over N across 8 cores; per core SHARD=62500
samples padded to 512*128=65536; sample s lives at (p=s//512, t=s%512)):

  Stage 1 (PE): per tile t (128 samples) and k-pair p in 0..3:
      Y^T[(kappa,j), n] = sum_i Mpair_p[i, (kappa,j)] * rowT[i, n]
    i.e. matmul(lhsT=Mquad[:,p,:] [64,128], rhs=rowT_g[:,t,:] [64,128])
    -> PSUM f32 [128, 128], four pairs packed in one [128,512] bank slice.

  Col-multiply U^T = Y^T * colT2 (colT duplicated on both partition halves),
  split across three engines per 2-tile block:
      ACT   : bridge pairs 0-1  PSUM f32 -> SBUF bf16
      DVE   : bf16 mult pairs 0-1; f32 PSUM-direct mult pair 2
      Pool  : f32 PSUM-direct mult pair 3

  Stage 2 (PE): reduce over j=64 per kappa via constant selection matrix:
      matmul(lhsT=U^T[:,b,pair,:] [128,128], rhs=sel [128,2])
    -> rec PSUM [128, 2] slices of a [128,64,8] bank (k = 2*pair+kappa).

  ACT: sigmoid per 64-tile group [128,64,8] -> SBUF f32; DMA out.
"""

import sys

sys.path.insert(0, "/opt/trn_rl_repo")

import numpy as np
import ml_dtypes

import concourse.bass as bass
import concourse.bacc as bacc
import concourse.mybir as mybir
from concourse import tile
from concourse.bass_utils import run_bass_kernel_spmd

N, D, R = 500000, 64, 8
NCORES = 8
SHARD = N // NCORES            # 62500
TPP = 504                      # samples per partition; 504*128 = 64512 >= 62500
SHARD_PAD = TPP * 128
W = 63                         # tiles per group (rec PSUM bank = [128,63,8] f32)
NGROUPS = TPP // W             # 8
BF16 = mybir.dt.bfloat16
F32 = mybir.dt.float32

_CACHE: dict = {}


def _build_program():
    if "nc" in _CACHE:
        return _CACHE["nc"]

    nc = bacc.Bacc(
        "TRN2", target_bir_lowering=False, debug=False, num_devices=NCORES
    )

    rowT_d = nc.dram_tensor("rowt", [D, TPP, 128], BF16, kind="ExternalInput")
    colT_d = nc.dram_tensor("colt", [D, TPP, 128], BF16, kind="ExternalInput")
    mq_d = nc.dram_tensor("mquad", [D, 4 * 128], BF16, kind="ExternalInput")
    sel_d = nc.dram_tensor("sel", [128, 2], BF16, kind="ExternalInput")
    out_d = nc.dram_tensor("out", [SHARD_PAD, R], F32, kind="ExternalOutput")

    out_v = out_d.ap().rearrange("(p t) k -> p t k", p=128)

    MULT = mybir.AluOpType.mult

    with tile.TileContext(nc) as tc:
        with (
            tc.tile_pool(name="const", bufs=1) as cpool,
            tc.tile_pool(name="io", bufs=2) as iopool,
            tc.tile_pool(name="work", bufs=4) as wpool,
            tc.tile_pool(name="psum_y", bufs=2, space="PSUM") as py_pool,
            tc.tile_pool(name="psum_r", bufs=2, space="PSUM") as pr_pool,
        ):
            mquad = cpool.tile([D, 4, 128], BF16, tag="mquad")
            sel = cpool.tile([128, 2], BF16, tag="sel")
            nc.sync.dma_start(
                mquad[:].rearrange("d p j -> d (p j)"), mq_d.ap()
            )
            nc.sync.dma_start(sel[:], sel_d.ap())

            for g in range(NGROUPS):
                t0 = g * W
                rowT_g = iopool.tile([D, W, 128], BF16, tag="rowT_g")
                colT2_g = iopool.tile([128, W, 128], BF16, tag="colT2_g")
                nc.sync.dma_start(rowT_g[:], rowT_d.ap()[:, t0 : t0 + W, :])
                # colT duplicated onto both partition halves (j and j+64)
                nc.sync.dma_start(
                    colT2_g[0:64, :, :], colT_d.ap()[:, t0 : t0 + W, :]
                )
                nc.sync.dma_start(
                    colT2_g[64:128, :, :], colT_d.ap()[:, t0 : t0 + W, :]
                )

                rec_ps = pr_pool.tile([128, W, R], F32, tag="rec")

                for b0 in range(0, W, 3):
                    # stage 1: 12 matmuls -> Y^T for 3 tiles in one PSUM tile
                    y_ps = py_pool.tile([128, 3, 4, 128], F32, tag="y")
                    for b in range(3):
                        t = b0 + b
                        for p in range(4):
                            nc.tensor.matmul(
                                y_ps[:, b, p, :],
                                mquad[:, p, :],
                                rowT_g[:, t, :],
                            )

                    colb = colT2_g[:, b0 : b0 + 3, :]
                    ut = wpool.tile([128, 3, 4, 128], BF16, tag="ut")

                    # ACT: bridge pairs 1-3 to bf16 (GPSIMD cannot read PSUM)
                    ybf = wpool.tile([128, 3, 3, 128], BF16, tag="ybf")
                    nc.scalar.copy(ybf[:], y_ps[:, :, 1:4, :])
                    # DVE: f32 PSUM-direct mult pair 0
                    nc.vector.tensor_tensor(
                        out=ut[:, :, 0, :],
                        in0=y_ps[:, :, 0, :],
                        in1=colb,
                        op=MULT,
                    )
                    # DVE: bf16 mult pairs 1-2
                    nc.vector.tensor_tensor(
                        out=ut[:, :, 1:3, :],
                        in0=ybf[:, :, 0:2, :],
                        in1=colb.unsqueeze(2).broadcast_to([128, 3, 2, 128]),
                        op=MULT,
                    )
                    # Pool: bf16 mult pair 3 (SBUF only)
                    nc.gpsimd.tensor_tensor(
                        out=ut[:, :, 3, :],
                        in0=ybf[:, :, 2, :],
                        in1=colb,
                        op=MULT,
                    )

                    # stage 2: PE reduce over j via selection matrix
                    for b in range(3):
                        t = b0 + b
                        for p in range(4):
                            nc.tensor.matmul(
                                rec_ps[:, t, 2 * p : 2 * p + 2],
                                ut[:, b, p, :],
                                sel[:],
                            )

                sig_g = wpool.tile([128, W, R], F32, tag="sig")
                nc.scalar.activation(
                    sig_g[:],
                    rec_ps[:],
                    mybir.ActivationFunctionType.Sigmoid,
                )
                nc.sync.dma_start(out_v[:, t0 : t0 + W, :], sig_g[:])

    nc.compile()
    _CACHE["nc"] = nc
    return nc


def _prep_inputs(inputs_row, inputs_col, global_interaction, local_variation):
    d = np.asarray(local_variation, np.float32)
    g = np.asarray(global_interaction, np.float32)
    # Mquad[i, p, (kappa, j)] = M_{2p+kappa}[i, j] = d[k,i]*G[i,j]*d[k,j]
    mk = np.einsum("ki,ij,kj->kij", d, g, d)            # [8, 64, 64]
    mquad = (
        mk.reshape(4, 2, D, D)
        .transpose(2, 0, 1, 3)                           # [i, p, kappa, j]
        .reshape(D, 4 * 128)
    )
    mquad = np.ascontiguousarray(mquad).astype(ml_dtypes.bfloat16)
    sel = np.zeros((128, 2), np.float32)
    sel[0:64, 0] = 1.0
    sel[64:128, 1] = 1.0
    sel = sel.astype(ml_dtypes.bfloat16)

    pad = SHARD_PAD - SHARD
    in_maps = []
    for c in range(NCORES):
        sl = slice(c * SHARD, (c + 1) * SHARD)
        rr = np.concatenate(
            [np.asarray(inputs_row[sl], np.float32), np.zeros((pad, D), np.float32)]
        ).astype(ml_dtypes.bfloat16)
        cc = np.concatenate(
            [np.asarray(inputs_col[sl], np.float32), np.zeros((pad, D), np.float32)]
        ).astype(ml_dtypes.bfloat16)
        rowt = np.ascontiguousarray(rr.reshape(128, TPP, D).transpose(2, 1, 0))
        colt = np.ascontiguousarray(cc.reshape(128, TPP, D).transpose(2, 1, 0))
        in_maps.append(
            {"rowt": rowt, "colt": colt, "mquad": mquad, "sel": sel}
        )
    return in_maps


def kernel(inputs_row, inputs_col, global_interaction, local_variation):
    nc = _build_program()
    in_maps = _prep_inputs(
        inputs_row, inputs_col, global_interaction, local_variation
    )
    res = run_bass_kernel_spmd(nc, in_maps, list(range(NCORES)))
    outs = [res.results[c]["out"][:SHARD] for c in range(NCORES)]
    full = np.concatenate(outs, axis=0)  # [N, 8] f32
    return np.ascontiguousarray(full.T)  # [8, N]


if __name__ == "__main__":
    rng = np.random.default_rng(0)
    inputs = {
        "inputs_row": rng.standard_normal((N, D), dtype=np.float32),
        "inputs_col": rng.standard_normal((N, D), dtype=np.float32),
        "global_interaction": rng.uniform(-0.2, 0.2, (D, D)).astype(np.float32),
        "local_variation": rng.uniform(-0.3, 0.3, (R, D)).astype(np.float32),
    }
    out = kernel(**inputs)
    print("out", out.shape, out.dtype, out[:, :3])
